# revision 1
# baseline (speedup 1.0000x reference)
"""Spectral pooling (FFT2 -> crop low freqs -> IFFT2) as dense DFT matmuls on TRN2.

Input  x: (32, 256, 64, 64) fp32  -- channels 0:128 real part, 128:256 imag part
Output y: (32, 256, 32, 32) fp32

Math: per complex image X (64x64), Y = A @ X @ A.T with
  A = sqrt(1/(64*32)) * IDFT32 @ Crop @ DFT64   (32x64 complex)
Sharding: batch dim across 8 cores (4 batches/core), no communication.

HBM I/O is bf16 and pre-packed on the host into the exact SBUF layouts so
every DMA moves >=4KB contiguous runs per partition at full bus rate:
  x_dev[b]  [128, 8192] = x[b] with partitions (xc, h), cols (c, w)
  y_dev[b]  [128, 2048] = raw stage-2 results; host unscrambles + upcasts.
(The fp32->bf16 cast is numerically identical to the in-DMA cast the previous
revision used; matmuls consumed bf16 either way.)

Per complex-channel pair (jj half of a quad q) stage 1 computes, in ONE
matmul, P = A X for two images j=0,1 (complex combine happens inside the
K contraction -- partitions hold (xc, h), the moving R1 = [[ArT,AiT],
[-AiT,ArT]] streams only 64 cols):
  psum1[(j,w), (pc,p)] = P^j_pc[p,w]
Stage 2 accumulates 2 matmuls (Pr-slice x D2r + Pi-slice x D2i, block-diag
over j) -> psum2[(jj,p), (j,yc,p2)] = Y. 96 streamed cols/image vs 128 for
the 2-matmul-per-stage scheme.

PSUM->SBUF copies are batched into full 2KB banks ([128,512]) and split
across DVE and ACT so neither exceeds the DMA bottleneck (~29us/rep).
"""

import math

import numpy as np

from concourse import bass, mybir
from concourse.bass_utils import run_bass_kernel_spmd
from concourse.tile import TileContext

N_CORES = 8
B_FULL, C2, H, W = 32, 256, 64, 64
HP, WP = 32, 32
BPC = B_FULL // N_CORES  # batches per core

F32 = mybir.dt.float32
BF16 = mybir.dt.bfloat16


def _split_multi_waits(nc):
    """This walrus build rejects instructions carrying more than one semaphore
    wait. Hoist extra waits onto same-engine NOPs inserted just before the
    instruction (engine queues execute in order, so blocking is equivalent)."""
    n_split = 0
    for f in nc.m.functions:
        for bb in f.blocks:
            insts = bb.instructions
            out = []
            for inst in insts:
                si = inst.sync_info
                waits = list(si.on_wait) if si and si.on_wait else []
                if len(waits) > 1:
                    si.on_wait = waits[-1:]
                    for w in waits[:-1]:
                        nop = mybir.InstNoOp(
                            name=nc.get_next_instruction_name(),
                            ins=[],
                            outs=[],
                            engine=inst.engine,
                            sync_info=mybir.SyncInfo(on_wait=[w], on_update=[]),
                        )
                        out.append(nop)
                        n_split += 1
                out.append(inst)
            if len(out) != len(insts):
                insts[:] = out
    return n_split


def _a_matrix():
    topf = int(math.ceil(H * 0.5 / 2))  # 16
    midf = H // 2 + topf  # 48
    F = np.exp(-2j * np.pi * np.outer(np.arange(H), np.arange(H)) / H)
    G = np.exp(2j * np.pi * np.outer(np.arange(HP), np.arange(HP)) / HP)
    keep = list(range(topf)) + list(range(midf, H))
    S = np.zeros((HP, H))
    S[np.arange(HP), keep] = 1
    return (G @ S @ F) / np.sqrt(H * W * HP * WP) ** 0.5


def _dft_constants():
    """[128, 320] f32: R1 (64 cols) | D2r (128) | D2i (128)."""
    A = _a_matrix()
    ArT = A.real.astype(np.float32).T  # [64, 32]
    AiT = A.imag.astype(np.float32).T

    R1 = np.block([[ArT, AiT], [-AiT, ArT]])  # [128(xc,h), 64(pc,p)]
    C2r = np.concatenate([ArT, AiT], axis=1)  # [64(w), 64(yc,p2)]
    C2i = np.concatenate([-AiT, ArT], axis=1)
    D2r = np.zeros((128, 128), np.float32)
    D2i = np.zeros((128, 128), np.float32)
    D2r[:64, :64] = C2r
    D2r[64:, 64:] = C2r
    D2i[:64, :64] = C2i
    D2i[64:, 64:] = C2i
    return np.concatenate([R1, D2r, D2i], axis=1)


def build_program(reps: int = 1, split_waits: bool = True,
                  loop_n: int | None = None, probe: str | None = None,
                  p1b: int = 3, p2b: int = 2, s1b: int = 3, inb: int = 2,
                  outb: int = 2):
    """reps > 1 unrolls the whole pipeline in-NEFF over the same data so the
    marginal cost per rep can be measured without the ~65ms axon dispatch
    overhead. loop_n wraps the unrolled body in a hardware For_i loop so the
    device time can be made to dominate host dispatch jitter entirely."""
    nc = bass.Bass("TRN2", target_bir_lowering=False, debug=False)
    x = nc.dram_tensor("x", [BPC, 128, 8192], BF16, kind="ExternalInput").ap()
    dm = nc.dram_tensor("dmats", [128, 320], BF16, kind="ExternalInput").ap()
    y = nc.dram_tensor("y", [BPC, 128, 2048], BF16, kind="ExternalOutput").ap()

    with TileContext(nc) as tc:
        with (
            tc.tile_pool(name="consts", bufs=1) as cpool,
            tc.tile_pool(name="inp", bufs=inb) as ipool,
            tc.tile_pool(name="sb1", bufs=s1b) as s1pool,
            tc.tile_pool(name="sbout", bufs=outb) as opool,
            tc.tile_pool(name="ps1", bufs=p1b, space="PSUM") as p1pool,
            tc.tile_pool(name="ps2", bufs=p2b, space="PSUM") as p2pool,
        ):
            dmb = cpool.tile([128, 320], BF16, tag="dmb")
            nc.sync.dma_start(out=dmb, in_=dm)
            r1 = dmb[:, 0:64]
            d2r = dmb[:, 64:192]
            d2i = dmb[:, 192:320]

            def emit_body():
                for b in [b for _ in range(reps) for b in range(BPC)]:
                    emit_batch(b)

            def emit_batch(b):
                tin = ipool.tile([128, 8192], BF16, tag="tin")
                if probe == "nodma":
                    nc.gpsimd.memset(tin[:, 0:8], 0.0)
                elif probe == "dma2q":
                    nc.gpsimd.dma_start(out=tin[:, 0:4096], in_=x[b][:, 0:4096])
                    nc.sync.dma_start(out=tin[:, 4096:], in_=x[b][:, 4096:])
                else:
                    nc.gpsimd.dma_start(out=tin, in_=x[b])
                sb_out = opool.tile([128, 2048], BF16, tag="sb_out")
                if probe in ("dmaonly", "dma2q"):
                    nc.vector.memset(sb_out[:, 0:8], 0.0)
                    nc.sync.dma_start(out=y[b], in_=sb_out)
                    return
                ps2 = None
                for sg in range(8):  # stage-1 groups of 4 quads
                    ps1 = p1pool.tile([128, 512], F32, tag="ps1")
                    for q4 in range(4):
                        q = 4 * sg + q4
                        for jj in range(2):
                            lo = 256 * q + 128 * jj
                            nc.tensor.matmul(
                                out=ps1[:, 128 * q4 + 64 * jj :
                                        128 * q4 + 64 * jj + 64],
                                lhsT=tin[:, lo : lo + 128],
                                rhs=r1,
                                start=True,
                                stop=True,
                                tile_position=(0, 0),
                            )
                    # copy out deinterleaving pc: ps1 cols (q,jj,pc,pp) ->
                    # sb1 cols (q,pc,jj,pp), one 3D-AP copy per pc so the
                    # stage-2 lhsT slices below are contiguous (the PE and
                    # ACT ISAs reject >1-free-dim / >3D APs respectively).
                    sb1 = s1pool.tile([128, 512], BF16, tag="sb1")
                    ps1v = ps1.rearrange(
                        "z (q jj pc pp) -> z q jj pc pp", q=4, jj=2, pc=2, pp=32
                    )
                    sb1v = sb1.rearrange(
                        "z (q pc jj pp) -> z q pc jj pp", q=4, pc=2, jj=2, pp=32
                    )
                    eng = nc.vector if sg % 2 == 0 else nc.scalar
                    for pc in range(2):
                        if sg % 2 == 0:
                            eng.tensor_copy(
                                out=sb1v[:, :, pc], in_=ps1v[:, :, :, pc]
                            )
                        else:
                            eng.copy(out=sb1v[:, :, pc], in_=ps1v[:, :, :, pc])
                    if sg % 2 == 0:
                        ps2 = p2pool.tile([128, 512], F32, tag="ps2")
                    for q4 in range(4):
                        qpar = q4 % 2
                        slot = 2 * (sg % 2) + q4 // 2
                        out_ap = ps2[64 * qpar : 64 * qpar + 64,
                                     128 * slot : 128 * slot + 128]
                        nc.tensor.matmul(
                            out=out_ap,
                            lhsT=sb1[:, 128 * q4 : 128 * q4 + 64],
                            rhs=d2r,
                            start=True,
                            stop=False,
                            tile_position=(0, 64 * qpar),
                        )
                        nc.tensor.matmul(
                            out=out_ap,
                            lhsT=sb1[:, 128 * q4 + 64 : 128 * q4 + 128],
                            rhs=d2i,
                            start=False,
                            stop=True,
                            tile_position=(0, 64 * qpar),
                        )
                    if sg % 2 == 1:
                        sgp = sg // 2
                        o = sb_out[:, 512 * sgp : 512 * sgp + 512]
                        if sgp % 2 == 0:
                            nc.scalar.copy(out=o, in_=ps2)
                        else:
                            nc.vector.tensor_copy(out=o, in_=ps2)
                if probe != "nodma":
                    nc.sync.dma_start(out=y[b], in_=sb_out)

            if loop_n is None:
                emit_body()
            else:
                with tc.For_i(0, loop_n):
                    emit_body()
    if split_waits:
        _split_multi_waits(nc)
    return nc


def _bf16(a: np.ndarray) -> np.ndarray:
    return a.astype(mybir.dt.np(BF16))


def _pack_x(x_shard: np.ndarray) -> np.ndarray:
    """[BPC, 256, 64, 64] f32 -> [BPC, 128, 8192] bf16, partitions (xc, h),
    cols (c, w)."""
    b = x_shard.shape[0]
    xr = x_shard.reshape(b, 2, 128, 64, 64).transpose(0, 1, 3, 2, 4)
    return _bf16(np.ascontiguousarray(xr).reshape(b, 128, 8192))


def _unpack_y(y_dev: np.ndarray) -> np.ndarray:
    """[BPC, 128, 2048] bf16 -> [BPC, 256, 32, 32] f32.

    part = 64*qpar + 32*jj + p ; col = 512*sgp + 128*(2*sh+sl) + 64*j
    + 32*yc + p2 ; channel = 128*yc + 32*sgp + 16*sh + 8*sl + 4*qpar
    + 2*jj + j."""
    b = y_dev.shape[0]
    a = y_dev.astype(np.float32).reshape(b, 2, 2, 32, 4, 2, 2, 2, 2, 32)
    #                                       qpar jj p sgp sh sl j yc p2
    a = a.transpose(0, 8, 4, 5, 6, 1, 2, 7, 3, 9)  # b yc sgp sh sl qpar jj j p p2
    return np.ascontiguousarray(a).reshape(b, 256, 32, 32)


def _make_in_map(x_shard: np.ndarray, dmats: np.ndarray) -> dict:
    return {"x": _pack_x(x_shard), "dmats": _bf16(dmats)}


_CACHED = {}


def _get_program():
    if "nc" not in _CACHED:
        _CACHED["nc"] = build_program()
        _CACHED["consts"] = _dft_constants()
    return _CACHED["nc"], _CACHED["consts"]


def kernel(x: np.ndarray) -> np.ndarray:
    assert x.shape == (B_FULL, C2, H, W) and x.dtype == np.float32
    nc, dmats = _get_program()
    in_maps = [
        _make_in_map(x[BPC * k : BPC * (k + 1)], dmats)
        for k in range(N_CORES)
    ]
    res = run_bass_kernel_spmd(nc, in_maps, list(range(N_CORES)))
    out = np.concatenate(
        [_unpack_y(np.asarray(res.results[k]["y"])) for k in range(N_CORES)],
        axis=0,
    )
    return out.astype(np.float32, copy=False)


if __name__ == "__main__":
    rng = np.random.default_rng(0)
    x = rng.standard_normal((B_FULL, C2, H, W)).astype(np.float32)
    y = kernel(x)
    print("kernel output", y.shape, y.dtype)



# revision 33
# speedup vs baseline: 6.5531x; 6.5531x over previous
"""Spectral pooling (FFT2 -> crop low freqs -> IFFT2) as dense DFT matmuls on TRN2.

Input  x: (32, 256, 64, 64) fp32  -- channels 0:128 real part, 128:256 imag part
Output y: (32, 256, 32, 32) fp32

Math: per complex image X (64x64), Y = A @ X @ A.T with
  A = sqrt(1/(64*32)) * IDFT32 @ Crop @ DFT64   (32x64 complex)
Sharding: batch dim across 8 cores (4 batches/core), no communication.

HBM I/O is bf16 and pre-packed on the host into the exact SBUF layouts so
every DMA moves >=4KB contiguous runs per partition at full bus rate:
  x_dev[b]  [128, 8192] = x[b] with partitions (xc, h), cols (c, w)
  y_dev[b]  [128, 2048] = raw stage-2 results; host unscrambles + upcasts.

PE cost on TRN2 is the number of MOVING-operand columns streamed (bf16
1 col/cycle; LDWEIGHTS overlaps with streaming). Stage 1 keeps the data
chunk stationary and streams the 64-col combined-complex DFT matrix R1 =
[[ArT,AiT],[-AiT,ArT]]: per 2 images one matmul streaming 64 cols ->
psum1[(j,w), (jj,pc,pp)] = P = A X. Stage 2 is flipped vs. the obvious
scheme: D2r/D2i become the STATIONARY operand and the sb1 data slices
stream, in two phases per 2-supergroup so the stationary only changes
twice per 8 quads:
  phase r: out[(j,yc,n2), (jj,n1)] += D2r^T @ sb1[pc=0 slice]   (8 quads)
  phase i: out                      += D2i^T @ sb1[pc=1 slice]
This halves stage-2 streamed cols (64/quad/phase vs 256/quad) -> total
8192 streamed cols per batch ~= the 23.4us/core HBM-in roofline (ridge).

Input is DMAed as 8 tiles of [128, 4096] (8KB packets, ~330GB/s on the
HWDGE path), all issued up-front on the sync-engine HWDGE queue (sync is
otherwise idle, so the tile list-scheduler keeps the issues prompt; busy
engines like ACT get their DMA issues reordered behind compute, and
>8 concurrent DMA instructions serialize on the 8 DMAHW proc
semaphores). Outputs go per-batch on the same queue; dmats rides the
scalar queue head. PSUM->SBUF copies are split across DVE and ACT.
Measured single-shot NEFF exec (NTFF): ~47.4us vs 58.2us for the
previous stage-2 data-stationary revision."""

import math

import numpy as np

from concourse import bass, mybir
from concourse.bass_utils import run_bass_kernel_spmd
from concourse.tile import TileContext

N_CORES = 8
B_FULL, C2, H, W = 32, 256, 64, 64
HP, WP = 32, 32
BPC = B_FULL // N_CORES  # batches per core
NG = 4  # 2-sg groups per batch (each group = 2048 input cols = 32 images)

F32 = mybir.dt.float32
BF16 = mybir.dt.bfloat16


def _split_multi_waits(nc):
    """This walrus build rejects instructions carrying more than one semaphore
    wait. Hoist extra waits onto same-engine NOPs inserted just before the
    instruction (engine queues execute in order, so blocking is equivalent)."""
    n_split = 0
    for f in nc.m.functions:
        for bb in f.blocks:
            insts = bb.instructions
            out = []
            for inst in insts:
                si = inst.sync_info
                waits = list(si.on_wait) if si and si.on_wait else []
                if len(waits) > 1:
                    si.on_wait = waits[-1:]
                    for w in waits[:-1]:
                        nop = mybir.InstNoOp(
                            name=nc.get_next_instruction_name(),
                            ins=[],
                            outs=[],
                            engine=inst.engine,
                            sync_info=mybir.SyncInfo(on_wait=[w], on_update=[]),
                        )
                        out.append(nop)
                        n_split += 1
                out.append(inst)
            if len(out) != len(insts):
                insts[:] = out
    return n_split


def _a_matrix():
    topf = int(math.ceil(H * 0.5 / 2))  # 16
    midf = H // 2 + topf  # 48
    F = np.exp(-2j * np.pi * np.outer(np.arange(H), np.arange(H)) / H)
    G = np.exp(2j * np.pi * np.outer(np.arange(HP), np.arange(HP)) / HP)
    keep = list(range(topf)) + list(range(midf, H))
    S = np.zeros((HP, H))
    S[np.arange(HP), keep] = 1
    return (G @ S @ F) / np.sqrt(H * W * HP * WP) ** 0.5


def _dft_constants():
    """[128, 320] f32: R1 (64 cols) | D2r (128) | D2i (128)."""
    A = _a_matrix()
    ArT = A.real.astype(np.float32).T  # [64, 32]
    AiT = A.imag.astype(np.float32).T

    R1 = np.block([[ArT, AiT], [-AiT, ArT]])  # [128(xc,h), 64(pc,p)]
    C2r = np.concatenate([ArT, AiT], axis=1)  # [64(w), 64(yc,p2)]
    C2i = np.concatenate([-AiT, ArT], axis=1)
    D2r = np.zeros((128, 128), np.float32)
    D2i = np.zeros((128, 128), np.float32)
    D2r[:64, :64] = C2r
    D2r[64:, 64:] = C2r
    D2i[:64, :64] = C2i
    D2i[64:, 64:] = C2i
    return np.concatenate([R1, D2r, D2i], axis=1)


def build_program(reps: int = 1, split_waits: bool = True,
                  loop_n: int | None = None,
                  p1b: int = 4, p2b: int = 3, s1b: int = 6,
                  outb: int = 2, in_eng: str = "sync", in_cols: int = 4096,
                  out_eng: str = "sync", out_split: int = 1):
    """reps > 1 unrolls the whole pipeline in-NEFF over the same data so the
    marginal cost per rep can be measured without the ~65ms axon dispatch
    overhead. loop_n wraps the unrolled body in a hardware For_i loop."""
    nc = bass.Bass("TRN2", target_bir_lowering=False, debug=False)
    x = nc.dram_tensor("x", [BPC, 128, 8192], BF16, kind="ExternalInput").ap()
    dm = nc.dram_tensor("dmats", [128, 320], BF16, kind="ExternalInput").ap()
    y = nc.dram_tensor("y", [BPC, 128, 2048], BF16, kind="ExternalOutput").ap()

    with TileContext(nc) as tc:
        with (
            tc.tile_pool(name="consts", bufs=1) as cpool,
            tc.tile_pool(name="inp", bufs=(4 if in_eng == "g2tail"
                                           else BPC * (8192 // in_cols))) as ipool,
            tc.tile_pool(name="inpB", bufs=3) as ipoolB,
            tc.tile_pool(name="sb1", bufs=s1b) as s1pool,
            tc.tile_pool(name="sbout", bufs=outb) as opool,
            tc.tile_pool(name="ps1", bufs=p1b, space="PSUM") as p1pool,
            tc.tile_pool(name="ps2", bufs=p2b, space="PSUM") as p2pool,
        ):
            dmb = cpool.tile([128, 320], BF16, tag="dmb")
            nc.scalar.dma_start(out=dmb, in_=dm)
            r1 = dmb[:, 0:64]
            d2r = dmb[:, 64:192]
            d2i = dmb[:, 192:320]

            def emit_body():
                for _ in range(reps):
                    emit_rep()

            sb_outs = {}

            def emit_rep():
                in_engines = {
                    "sync": [nc.sync],
                    "scalar": [nc.scalar],
                    "gpsimd": [nc.gpsimd],
                    "mix": [nc.sync, nc.gpsimd],
                    "hwmix": [nc.sync, nc.scalar],
                    # scalar gets only the first two issues (they come before
                    # any ACT copy in its stream, so the list scheduler can't
                    # push them behind compute); the rest ride on sync.
                    "front2": [nc.scalar, nc.scalar, nc.sync],
                    "tail3": [nc.sync],
                    "g2tail": [nc.sync],
                    "taper": [nc.sync],
                }[in_eng]
                # prefetch the entire input up-front, alternating queues
                tins = {}
                if in_eng == "taper":
                    # b0 as two fine tiles (early PE start), b1-b3 as whole
                    # 16KB-packet tiles, all on the sync HWDGE queue
                    for hb in range(2):
                        t = ipool.tile([128, 4096], BF16, tag="tin", name="tin")
                        nc.sync.dma_start(
                            out=t, in_=x[0][:, 4096 * hb : 4096 * (hb + 1)]
                        )
                        for k in range(2):
                            tins[(0, 2 * hb + k)] = t[:, 2048 * k : 2048 * (k + 1)]
                    for b in range(1, BPC):
                        t = ipoolB.tile([128, 8192], BF16, tag="tinB", name="tinB")
                        nc.sync.dma_start(out=t, in_=x[b])
                        for g in range(NG):
                            tins[(b, g)] = t[:, 2048 * g : 2048 * (g + 1)]
                elif in_eng == "g2tail":
                    # b0,b1 as fine tiles on sync HWDGE (early PE start);
                    # b2,b3 as whole-batch 16KB-packet tiles on gpsimd SWDGE
                    # (idle engine -> prompt issue, second descriptor stream)
                    for b in range(2):
                        for hb in range(2):
                            t = ipool.tile([128, 4096], BF16, tag="tin",
                                           name="tin")
                            nc.sync.dma_start(
                                out=t, in_=x[b][:, 4096 * hb : 4096 * (hb + 1)]
                            )
                            for k in range(2):
                                tins[(b, 2 * hb + k)] = t[:, 2048 * k :
                                                          2048 * (k + 1)]
                    for b in range(2, BPC):
                        t = ipoolB.tile([128, 8192], BF16, tag="tinB",
                                        name="tinB")
                        nc.gpsimd.dma_start(out=t, in_=x[b])
                        for g in range(NG):
                            tins[(b, g)] = t[:, 2048 * g : 2048 * (g + 1)]
                else:
                    tiles_per_b = 8192 // in_cols
                    gp_per_tile = in_cols // 2048
                    for b in range(BPC):
                        for hb in range(tiles_per_b):
                            t = ipool.tile([128, in_cols], BF16, tag="tin",
                                           name="tin")
                            i = b * tiles_per_b + hb
                            if in_eng == "front2":
                                eng = in_engines[min(i, 2)]
                            elif in_eng == "tail3":
                                eng = (nc.scalar
                                       if i >= BPC * tiles_per_b - 3 else nc.sync)
                            else:
                                eng = in_engines[i % len(in_engines)]
                            eng.dma_start(
                                out=t, in_=x[b][:, in_cols * hb : in_cols * (hb + 1)]
                            )
                            for k in range(gp_per_tile):
                                tins[(b, gp_per_tile * hb + k)] = t[
                                    :, 2048 * k : 2048 * (k + 1)
                                ]
                # stage 2 of group i runs after stage 1 of group i+1 so the
                # PE never stalls on the PSUM->SBUF deinterleave copies.
                pending = None
                for b in range(BPC):
                    for g in range(NG):
                        sb1s = emit_s1(tins[(b, g)])
                        if pending is not None:
                            emit_s2(*pending)
                        pending = (sb1s, b, g)
                emit_s2(*pending)

            def emit_s1(tin):
                # stage 1: per sg (4 quads), data stationary, stream r1
                sb1s = []
                for sh in range(2):
                    ps1 = p1pool.tile([128, 512], F32, tag="ps1")
                    for q4 in range(4):
                        for jj in range(2):
                            lo = 1024 * sh + 256 * q4 + 128 * jj
                            nc.tensor.matmul(
                                out=ps1[:, 128 * q4 + 64 * jj :
                                        128 * q4 + 64 * jj + 64],
                                lhsT=tin[:, lo : lo + 128],
                                rhs=r1,
                                start=True,
                                stop=True,
                                tile_position=(0, 0),
                            )
                    # deinterleave pc: ps1 cols (q,jj,pc,pp) -> sb1 cols
                    # (q,pc,jj,pp) so stage-2 rhs slices are contiguous.
                    sb1 = s1pool.tile([128, 512], BF16, tag="sb1")
                    ps1v = ps1.rearrange(
                        "z (q jj pc pp) -> z q jj pc pp", q=4, jj=2, pc=2, pp=32
                    )
                    sb1v = sb1.rearrange(
                        "z (q pc jj pp) -> z q pc jj pp", q=4, pc=2, jj=2, pp=32
                    )
                    for pc in range(2):
                        if sh == 0:
                            nc.vector.tensor_copy(
                                out=sb1v[:, :, pc], in_=ps1v[:, :, :, pc]
                            )
                        else:
                            nc.scalar.copy(out=sb1v[:, :, pc], in_=ps1v[:, :, :, pc])
                    sb1s.append(sb1)
                return sb1s

            def emit_s2(sb1s, b, g):
                # stage 2: D2r/D2i stationary, stream sb1 slices (2 phases)
                if g == 0:
                    sb_outs[b] = opool.tile(
                        [128, 2048], BF16, tag="sb_out", name="sb_out"
                    )
                ps2 = p2pool.tile([128, 512], F32, tag="ps2")
                for sh in range(2):
                    for q4 in range(4):
                        for phase, dmat in ((0, d2r), (1, d2i)):
                            nc.tensor.matmul(
                                out=ps2[:, 64 * (4 * sh + q4) :
                                        64 * (4 * sh + q4) + 64],
                                lhsT=dmat,
                                rhs=sb1s[sh][:, 128 * q4 + 64 * phase :
                                             128 * q4 + 64 * phase + 64],
                                start=(phase == 0),
                                stop=(phase == 1),
                                tile_position=(0, 0),
                            )
                o = sb_outs[b][:, 512 * g : 512 * g + 512]
                if g % 2 == 0:
                    nc.scalar.copy(out=o, in_=ps2)
                else:
                    nc.vector.tensor_copy(out=o, in_=ps2)
                if out_eng == "mix3s":
                    # y0-y2 ride the scalar HWDGE queue (data-gated anyway),
                    # keeping the sync queue's descriptor slots for input;
                    # only the latency-critical last y goes on idle sync.
                    oeng = nc.sync if b == BPC - 1 else nc.scalar
                else:
                    oeng = {"scalar": nc.scalar, "sync": nc.sync,
                            "gpsimd": nc.gpsimd}[out_eng]
                if out_split == 2:
                    if g == 1:
                        oeng.dma_start(out=y[b][:, 0:1024],
                                       in_=sb_outs[b][:, 0:1024])
                    elif g == NG - 1:
                        oeng.dma_start(out=y[b][:, 1024:2048],
                                       in_=sb_outs[b][:, 1024:2048])
                elif g == NG - 1:
                    oeng.dma_start(out=y[b], in_=sb_outs[b])

            if loop_n is None:
                emit_body()
            else:
                with tc.For_i(0, loop_n):
                    emit_body()
    if split_waits:
        _split_multi_waits(nc)
    return nc


def _bf16(a: np.ndarray) -> np.ndarray:
    return a.astype(mybir.dt.np(BF16))


def _pack_x(x_shard: np.ndarray) -> np.ndarray:
    """[BPC, 256, 64, 64] f32 -> [BPC, 128, 8192] bf16, partitions (xc, h),
    cols (c, w)."""
    b = x_shard.shape[0]
    xr = x_shard.reshape(b, 2, 128, 64, 64).transpose(0, 1, 3, 2, 4)
    return _bf16(np.ascontiguousarray(xr).reshape(b, 128, 8192))


def _unpack_y(y_dev: np.ndarray) -> np.ndarray:
    """[BPC, 128, 2048] bf16 -> [BPC, 256, 32, 32] f32.

    part = 64*j + 32*yc + n2 ; col = 512*g + 64*(4*sh+q4) + 32*jj + n1 ;
    channel = 128*yc + 32*g + 16*sh + 4*q4 + 2*jj + j."""
    b = y_dev.shape[0]
    a = y_dev.astype(np.float32).reshape(b, 2, 2, 32, 4, 2, 4, 2, 32)
    #                                       j yc n2  g sh q4 jj n1
    a = a.transpose(0, 2, 4, 5, 6, 7, 1, 8, 3)  # b yc g sh q4 jj j n1 n2
    return np.ascontiguousarray(a).reshape(b, 256, 32, 32)


def _make_in_map(x_shard: np.ndarray, dmats: np.ndarray) -> dict:
    return {"x": _pack_x(x_shard), "dmats": _bf16(dmats)}


_CACHED = {}


def _get_program():
    if "nc" not in _CACHED:
        _CACHED["nc"] = build_program()
        _CACHED["consts"] = _dft_constants()
    return _CACHED["nc"], _CACHED["consts"]


def kernel(x: np.ndarray) -> np.ndarray:
    assert x.shape == (B_FULL, C2, H, W) and x.dtype == np.float32
    nc, dmats = _get_program()
    in_maps = [
        _make_in_map(x[BPC * k : BPC * (k + 1)], dmats)
        for k in range(N_CORES)
    ]
    res = run_bass_kernel_spmd(nc, in_maps, list(range(N_CORES)))
    out = np.concatenate(
        [_unpack_y(np.asarray(res.results[k]["y"])) for k in range(N_CORES)],
        axis=0,
    )
    return out.astype(np.float32, copy=False)


if __name__ == "__main__":
    rng = np.random.default_rng(0)
    x = rng.standard_normal((B_FULL, C2, H, W)).astype(np.float32)
    y = kernel(x)
    print("kernel output", y.shape, y.dtype)
